# revision 8
# baseline (speedup 1.0000x reference)
import os
import numpy as np

import concourse.bass as bass
import concourse.bacc as bacc
import concourse.tile as tile
from concourse import mybir
from concourse import bass_utils

# Problem dims (hardcoded per contract)
B, I, H, O = 64, 256, 512, 2
NCORES = 8
BL = B // NCORES   # 8 batch rows per core
GS = 16            # steps per PSUM bank group (16*32 fp32 = one 2KB bank)
F32 = mybir.dt.float32
BF16 = mybir.dt.bfloat16
NPBF16 = mybir.dt.np(BF16)

_cache = {}


def _build(S):
    """Transposed-h RNN kernel for one core (BL=8 sequences).

    Layout: h kept ONLY as hT [128, 4*BL] bf16 (col block k = batch values of
    h[128k:128k+128]).  Per step: 16 matmuls with U 128x128 tiles stationary
    (bf16, 8 moving cols each) accumulate U^T@hT onto a PSUM bank slot
    pre-filled with wxT + bias by per-16-step-group precompute matmuls.
    tanh runs as two [128,16] ACTs writing the next hT directly.  Matmuls
    are phase-ordered (k01 chunks consumed first, c01 regions stopped first)
    so ACT latency pipelines under PE work across steps.

    PSUM rule: start=True resets the has_written bits BANK-WIDE, so exactly
    one start=True per bank (the first precompute matmul); every other
    matmul accumulates (first write to a bit-clear element stores).
    """
    assert S % GS == 0
    nsteps_cols = S * BL

    nc = bacc.Bacc("TRN2", target_bir_lowering=False, debug=False,
                   enable_asserts=False)

    # DRAM inputs (per-core). xt: [I, S*BL] bf16, col s*BL+b = x[b, s, :].
    xt = nc.dram_tensor("xt", [I, nsteps_cols], BF16, kind="ExternalInput").ap()
    u = nc.dram_tensor("U_w", [H, H], BF16, kind="ExternalInput").ap()
    w = nc.dram_tensor("W_w", [I, H], BF16, kind="ExternalInput").ap()
    v = nc.dram_tensor("V_w", [H, O], BF16, kind="ExternalInput").ap()
    biasb = nc.dram_tensor("biasb", [1, H], BF16, kind="ExternalInput").ap()
    hconst = nc.dram_tensor("hconst", [128, 10], BF16, kind="ExternalInput").ap()
    h0z = nc.dram_tensor("h0z", [128, 4 * BL], BF16, kind="ExternalInput").ap()
    out = nc.dram_tensor("out", [BL, O], F32, kind="ExternalOutput").ap()

    Tanh = mybir.ActivationFunctionType.Tanh
    Sigmoid = mybir.ActivationFunctionType.Sigmoid

    from contextlib import ExitStack
    with tile.TileContext(nc) as tc, ExitStack() as ctx:
        cpool = ctx.enter_context(tc.tile_pool(name="const", bufs=1))
        hpool = ctx.enter_context(tc.tile_pool(name="h", bufs=2))
        ppool = ctx.enter_context(tc.tile_pool(name="ps", bufs=3, space="PSUM"))
        opool = ctx.enter_context(tc.tile_pool(name="po", bufs=1, space="PSUM"))

        # ---- constants / weights ----
        u_sb = [cpool.tile([128, H], BF16, tag=f"u{k}", name=f"u{k}")
                for k in range(4)]
        for k in range(4):
            nc.sync.dma_start(u_sb[k][:], u[128 * k:128 * (k + 1), :])
        w_sb = [cpool.tile([128, H], BF16, tag=f"w{k}", name=f"w{k}")
                for k in range(2)]
        for k in range(2):
            nc.sync.dma_start(w_sb[k][:], w[128 * k:128 * (k + 1), :])
        v_sb = cpool.tile([128, 4 * O], BF16, tag="v", name="v")
        for k in range(4):
            nc.sync.dma_start(v_sb[:, O * k:O * (k + 1)],
                              v[128 * k:128 * (k + 1), :])
        bias_sb = cpool.tile([1, H], BF16, tag="bias", name="bias")
        nc.sync.dma_start(bias_sb[:], biasb[:, :])
        ones_sb = cpool.tile([1, GS * BL], BF16, tag="ones", name="ones")
        nc.vector.memset(ones_sb[:], 1.0)
        hc_sb = cpool.tile([128, 10], BF16, tag="hc", name="hc")
        nc.sync.dma_start(hc_sb[:], hconst[:, :])

        # ---- all of x upfront: [128, 2*S*BL] bf16, half k at col k*S*BL ----
        xf = cpool.tile([128, 2 * nsteps_cols], BF16, tag="xf", name="xf")
        XCH = 2048 if nsteps_cols >= 2048 else nsteps_cols  # cols per DMA
        for k in range(2):
            for c0 in range(0, nsteps_cols, XCH):
                nc.sync.dma_start(
                    xf[:, k * nsteps_cols + c0:k * nsteps_cols + c0 + XCH],
                    xt[128 * k:128 * (k + 1), c0:c0 + XCH])

        # ---- initial h ----
        h_prev = hpool.tile([128, 4 * BL], BF16, tag="h", name="h_init")
        nc.sync.dma_start(h_prev[:], h0z[:, :])

        ngroups = S // GS
        for g in range(ngroups):
            # Precompute bias + wxT for the 16 steps of group g into PSUM.
            # Two separate bank tiles (c01 / c23 halves) so each half-step's
            # tanh read doesn't false-serialize against the other half's
            # matmul writes.
            # padded to a full 2KB bank so the two halves never share a bank
            # (start=True resets has_written bank-wide).
            ph = [ppool.tile([128, GS, 2 * BL], F32, tag=f"ps{i}",
                             name=f"ps{i}_{g}",
                             padded_shape=[128, GS, 4 * BL]) for i in range(2)]
            for c in range(4):
                psh = ph[c // 2]
                cc = c % 2
                nc.tensor.matmul(
                    psh[:, :, BL * cc:BL * (cc + 1)],
                    bias_sb[:1, 128 * c:128 * (c + 1)],
                    ones_sb[:1, :],
                    start=(cc == 0), stop=False, skip_group_check=True)
                for k in range(2):
                    nc.tensor.matmul(
                        psh[:, :, BL * cc:BL * (cc + 1)],
                        w_sb[k][:, 128 * c:128 * (c + 1)],
                        xf[:, k * nsteps_cols + g * GS * BL:
                           k * nsteps_cols + (g + 1) * GS * BL],
                        start=False, stop=False, skip_group_check=True)

            for r in range(GS):
                s = g * GS + r
                h_cur = hpool.tile([128, 4 * BL], BF16, tag="h", name=f"h{s}")

                def mm(c, k, stop):
                    nc.tensor.matmul(
                        ph[c // 2][:, r, BL * (c % 2):BL * (c % 2 + 1)],
                        u_sb[k][:, 128 * c:128 * (c + 1)],
                        h_prev[:, BL * k:BL * (k + 1)],
                        start=False, stop=stop, skip_group_check=True)

                # ph1: c01 x k01 ; ph2: c01 x k23 (stops c0,c1) ;
                # ph3: c23 x k01 ; ph4: c23 x k23 (stops c2,c3)
                for c in (0, 1):
                    for k in (0, 1):
                        mm(c, k, stop=False)
                for c in (0, 1):
                    for k in (2, 3):
                        mm(c, k, stop=(k == 3))
                nc.scalar.activation(h_cur[:, 0:2 * BL], ph[0][:, r, :],
                                     Tanh)
                for c in (2, 3):
                    for k in (0, 1):
                        mm(c, k, stop=False)
                for c in (2, 3):
                    for k in (2, 3):
                        mm(c, k, stop=(k == 3))
                nc.scalar.activation(h_cur[:, 2 * BL:4 * BL], ph[1][:, r, :],
                                     Tanh)
                h_prev = h_cur

        # ---- output head: o = sigmoid(h V + V_b) ----
        pso = opool.tile([BL, O], F32, tag="pso", name="pso")
        nc.tensor.matmul(pso[:], hc_sb[:1, 0:BL], hc_sb[:1, BL:BL + O],
                         start=True, stop=False, skip_group_check=True)
        for k in range(4):
            nc.tensor.matmul(pso[:], h_prev[:, BL * k:BL * (k + 1)],
                             v_sb[:, O * k:O * (k + 1)],
                             start=False, stop=(k == 3),
                             skip_group_check=True)
        o_sb = cpool.tile([BL, O], F32, tag="osb", name="osb")
        nc.scalar.activation(o_sb[:], pso[:], Sigmoid)
        nc.sync.dma_start(out[:, :], o_sb[:])

    nc.compile()
    return nc


def _bf16(a):
    return np.asarray(a, dtype=np.float32).astype(NPBF16)


def kernel(x, W_w, W_b, U_w, U_b, V_w, V_b):
    x = np.asarray(x, dtype=np.float32)
    S = x.shape[1]
    key = ("nc", S)
    if key not in _cache:
        _cache[key] = _build(S)
    nc = _cache[key]

    bias = (np.asarray(W_b) + np.asarray(U_b)).astype(np.float32)  # [H]
    hconst = np.zeros((128, 10), dtype=np.float32)
    hconst[0, :BL] = 1.0
    hconst[0, BL:BL + O] = np.asarray(V_b, dtype=np.float32)

    shared = {
        "U_w": _bf16(U_w),
        "W_w": _bf16(W_w),
        "V_w": _bf16(V_w),
        "biasb": _bf16(bias.reshape(1, H)),
        "hconst": _bf16(hconst),
        "h0z": np.zeros((128, 4 * BL), dtype=NPBF16),
    }
    in_maps = []
    for c in range(NCORES):
        xc = x[c * BL:(c + 1) * BL]                      # [BL, S, I]
        xtc = np.ascontiguousarray(xc.transpose(2, 1, 0).reshape(I, S * BL))
        in_maps.append(dict(shared, xt=_bf16(xtc)))

    trace = os.environ.get("RNN_TRACE", "0") == "1"
    kw = {}
    if trace:
        kw = dict(trace=True, tmpdir=os.environ.get("RNN_TRACE_DIR") or None)
    res = bass_utils.run_bass_kernel_spmd(nc, in_maps, core_ids=list(range(NCORES)),
                                          **kw)
    _cache["last_result"] = res
    return np.concatenate([r["out"] for r in res.results], axis=0)
